# revision 1
# baseline (speedup 1.0000x reference)
"""Distributed AttentionGNNLSTM kernel for 8 Trainium2 NeuronCores.

Sharding strategy (per spec sharding_hint):
  - GAT branch: edges (incl. self-loops) sharded across the 8 cores; each
    core computes partial softmax denominators + partial weighted message
    sums with segment_sum over its edge shard, combined with psum
    (all-reduce) across cores.  Node features x / weights replicated.
  - Graph mean-pool: nodes sharded across cores, partial per-graph sums
    psum-reduced.
  - LSTM + MHA branch: data-parallel over the 64 graphs (8 per core).
  - Final FC: per-core on its 8 graphs; host concatenates the shards.

Numerics note: the reference subtracts a per-destination segment max inside
the edge softmax.  alpha = exp(e - m)/sum(exp(e - m)) == exp(e)/sum(exp(e))
exactly (the max cancels), and |e| <= ~3 here so exp is safe in f32; we skip
the segment_max pass (saves one full scatter over the edges).
"""

import numpy as np
import jax
import jax.numpy as jnp
from functools import partial

N, E, B, T = 50000, 500000, 64, 50
F_NODE, F_SEQ, HID, H1, NCLS = 128, 64, 64, 4, 2
EMB = 2 * HID
NHEAD = 4
NC = 8                      # cores
ES = (E + N) // NC          # edges per core (with self loops): 68750
NS = N // NC                # nodes per core: 6250
BS = B // NC                # graphs per core: 8

_compiled = None


def _gat_partial(h, asrc, adst, src, dst):
    """Partial GAT aggregation over this core's edge shard.

    h: [N, H, C]; asrc/adst: [N, H]; src/dst: [ES] int32.
    Returns partial numerator [N, H, C] and partial denominator [N, H].
    """
    e = asrc[src] + adst[dst]                 # [ES, H]
    e = jnp.where(e >= 0, e, 0.2 * e)         # leaky_relu(0.2)
    ee = jnp.exp(e)
    z = jax.ops.segment_sum(ee, dst, num_segments=N)
    num = jax.ops.segment_sum(ee[:, :, None] * h[src], dst, num_segments=N)
    return num, z


def _lstm_dir(seq, Wih, Whh, bih, bhh):
    """seq [T, b, F] -> [T, b, HID] (single direction)."""
    b = seq.shape[1]
    h0 = jnp.zeros((b, HID), seq.dtype)
    WihT = Wih.T
    WhhT = Whh.T
    bias = bih + bhh

    def step(carry, xt):
        h, c = carry
        g = xt @ WihT + h @ WhhT + bias
        i, f, gg, o = jnp.split(g, 4, axis=-1)
        c = jax.nn.sigmoid(f) * c + jax.nn.sigmoid(i) * jnp.tanh(gg)
        h = jax.nn.sigmoid(o) * jnp.tanh(c)
        return (h, c), h

    _, hs = jax.lax.scan(step, (h0, h0), seq)
    return hs


def _core_fn(x, src, dst, batch_sh, seq_sh, p):
    """Runs on each core under pmap. Returns this core's [BS, NCLS] shard."""
    cid = jax.lax.axis_index('c')

    # ---- GAT layer 1 (4 heads) ----
    h1 = (x @ p['gnn1_W'].T).reshape(N, H1, HID)
    asrc1 = (h1 * p['gnn1_att_src']).sum(-1)
    adst1 = (h1 * p['gnn1_att_dst']).sum(-1)
    num1, z1 = _gat_partial(h1, asrc1, adst1, src, dst)
    num1 = jax.lax.psum(num1, 'c')
    z1 = jax.lax.psum(z1, 'c')
    g1 = jax.nn.relu((num1 / z1[:, :, None]).reshape(N, H1 * HID) + p['gnn1_b'])

    # ---- GAT layer 2 (1 head) ----
    h2 = (g1 @ p['gnn2_W'].T).reshape(N, 1, HID)
    asrc2 = (h2 * p['gnn2_att_src']).sum(-1)
    adst2 = (h2 * p['gnn2_att_dst']).sum(-1)
    num2, z2 = _gat_partial(h2, asrc2, adst2, src, dst)
    num2 = jax.lax.psum(num2, 'c')
    z2 = jax.lax.psum(z2, 'c')
    g2 = jax.nn.relu((num2 / z2[:, :, None]).reshape(N, HID) + p['gnn2_b'])

    # ---- graph mean-pool over this core's node shard ----
    g2_sh = jax.lax.dynamic_slice(g2, (cid * NS, 0), (NS, HID))
    sums = jax.ops.segment_sum(g2_sh, batch_sh, num_segments=B)
    cnts = jax.ops.segment_sum(jnp.ones((NS,), g2.dtype), batch_sh,
                               num_segments=B)
    sums = jax.lax.psum(sums, 'c')
    cnts = jax.lax.psum(cnts, 'c')
    gnn_pooled = sums / jnp.maximum(cnts, 1.0)[:, None]          # [B, HID]

    # ---- bidirectional LSTM on this core's BS graphs ----
    seq_t = seq_sh.transpose(1, 0, 2)                            # [T, BS, F]
    hf = _lstm_dir(seq_t, p['lstm_Wih_f'], p['lstm_Whh_f'],
                   p['lstm_bih_f'], p['lstm_bhh_f'])
    hb = _lstm_dir(seq_t[::-1], p['lstm_Wih_b'], p['lstm_Whh_b'],
                   p['lstm_bih_b'], p['lstm_bhh_b'])[::-1]
    lstm_out = jnp.concatenate([hf, hb], -1).transpose(1, 0, 2)  # [BS, T, EMB]

    # ---- self multi-head attention ----
    qkv = lstm_out @ p['attn_in_w'].T + p['attn_in_b']
    q, k, v = jnp.split(qkv, 3, axis=-1)
    hd = EMB // NHEAD
    q = q.reshape(BS, T, NHEAD, hd).transpose(0, 2, 1, 3)
    k = k.reshape(BS, T, NHEAD, hd).transpose(0, 2, 1, 3)
    v = v.reshape(BS, T, NHEAD, hd).transpose(0, 2, 1, 3)
    att = jax.nn.softmax(
        jnp.einsum('bhqd,bhkd->bhqk', q, k) / jnp.sqrt(jnp.float32(hd)), -1)
    o = jnp.einsum('bhqk,bhkd->bhqd', att, v).transpose(0, 2, 1, 3)
    o = o.reshape(BS, T, EMB)
    attn_out = o @ p['attn_out_w'].T + p['attn_out_b']
    attn_pooled = attn_out.mean(axis=1)                          # [BS, EMB]

    # ---- head ----
    gnn_sh = jax.lax.dynamic_slice(gnn_pooled, (cid * BS, 0), (BS, HID))
    combined = jnp.concatenate([gnn_sh, attn_pooled], axis=1)
    return combined @ p['fc_w'].T + p['fc_b']                    # [BS, NCLS]


_PARAM_NAMES = [
    'gnn1_W', 'gnn1_att_src', 'gnn1_att_dst', 'gnn1_b',
    'gnn2_W', 'gnn2_att_src', 'gnn2_att_dst', 'gnn2_b',
    'lstm_Wih_f', 'lstm_Whh_f', 'lstm_bih_f', 'lstm_bhh_f',
    'lstm_Wih_b', 'lstm_Whh_b', 'lstm_bih_b', 'lstm_bhh_b',
    'attn_in_w', 'attn_in_b', 'attn_out_w', 'attn_out_b', 'fc_w', 'fc_b',
]


def _get_compiled():
    global _compiled
    if _compiled is None:
        _compiled = jax.pmap(_core_fn, axis_name='c',
                             devices=jax.devices()[:NC])
    return _compiled


def kernel(**inputs):
    x = np.asarray(inputs['x'], np.float32)
    edge_index = np.asarray(inputs['edge_index'])
    batch = np.asarray(inputs['batch'], np.int32)
    seq_x = np.asarray(inputs['seq_x'], np.float32)

    # Self loops (PyG GATConv) + edge sharding across cores.
    loop = np.arange(N, dtype=np.int64)
    src = np.concatenate([edge_index[0], loop]).astype(np.int32)
    dst = np.concatenate([edge_index[1], loop]).astype(np.int32)
    src_sh = src.reshape(NC, ES)
    dst_sh = dst.reshape(NC, ES)

    x_rep = np.broadcast_to(x, (NC,) + x.shape)
    batch_sh = batch.reshape(NC, NS)
    seq_sh = seq_x.reshape(NC, BS, T, F_SEQ)

    params = {k: np.asarray(inputs[k], np.float32) for k in _PARAM_NAMES}
    params_rep = {k: np.broadcast_to(v, (NC,) + v.shape)
                  for k, v in params.items()}

    out_sh = _get_compiled()(x_rep, src_sh, dst_sh, batch_sh, seq_sh,
                             params_rep)
    return np.asarray(out_sh).reshape(B, NCLS).astype(np.float32)
